# revision 5
# baseline (speedup 1.0000x reference)
"""Trainium2 Bass kernel for the 3-layer GCN (nn_DeepGCNConv).

8 cores SPMD. Tokens = degree-sorted nodes in 392 blocks of 128, blocks
snake-striped across cores. Tables in DRAM, bf16 row-major:
  T1'[u] = dinv[u]*(x@W1)[u]        (phase 0, per-core slice + AllGather)
  T2'[u] = ((dinv*relu(agg1)) @ W2)[u]   (layer 1 output, + AllGather)

Aggregation per dst block: PSUM[dst,hid] = sum_r P_r^T @ G_r + I^T @ self
  - G_r: 128 gathered table rows (in-edges src-sorted, lo/hi window split
    for the int16 index range), bf16 SWDGE gathers.
  - P_r: 0/1 scatter tiles, host-built, streamed from HBM (bf16).
  - all dinv scaling folded into the single ACT that drains PSUM.

Layer 3 + mean pool fused via host-built CA = C*A [64, NPAD]:
  pooledT[hid,64] = sum_b r2_b^T @ CA_b ; 32KB AllReduce; final linear
  out^T = Wf^T @ pooledT with Wf = W3 @ lin_w computed on device.
"""

import numpy as np

NCORES = 8
NBLK = 49
NPAD = NCORES * NBLK * 128          # 50176
PER_CORE = NBLK * 128               # 6272
WIN = 32768
HIBASE = NPAD - WIN                 # 17408
D = 128
NGR = 64                            # graphs
NCLS = 16
RCALL = 16                          # max rounds (x128 idx) per gather call


# ---------------------------------------------------------------------------
# host preprocessing
# ---------------------------------------------------------------------------

def _preprocess(x, edge_index, batch):
    N = x.shape[0]
    src = np.asarray(edge_index[0], dtype=np.int64)
    dst = np.asarray(edge_index[1], dtype=np.int64)
    batch = np.asarray(batch, dtype=np.int64)

    deg = np.bincount(dst, minlength=N) + 1
    dinv = 1.0 / np.sqrt(deg.astype(np.float64))

    order = np.argsort(-deg, kind="stable")
    nblocks = NCORES * NBLK
    bi = np.arange(nblocks)
    r_, k_ = bi // NCORES, bi % NCORES
    core_of_blk = np.where(r_ % 2 == 0, k_, NCORES - 1 - k_)
    slot_of_blk = r_

    blk_sorted = np.arange(nblocks).repeat(128)[:N]
    pos_sorted = np.tile(np.arange(128), nblocks)[:N]
    tok_sorted = (core_of_blk[blk_sorted] * PER_CORE
                  + slot_of_blk[blk_sorted] * 128 + pos_sorted)
    tok_of_node = np.empty(N, np.int64)
    tok_of_node[order] = tok_sorted

    dinv_tok = np.zeros(NPAD)
    dinv_tok[tok_of_node] = dinv
    bat_tok = np.full(NPAD, 0, np.int64)
    bat_tok[tok_of_node] = batch
    real_tok = np.zeros(NPAD, bool)
    real_tok[tok_of_node] = True

    s_tok = tok_of_node[src]
    d_tok = tok_of_node[dst]

    xt = np.zeros((NCORES, D, PER_CORE), np.float32)
    xf = np.asarray(x, np.float32)
    core_of_node = tok_of_node // PER_CORE
    for c in range(NCORES):
        sel = core_of_node == c
        xt[c][:, tok_of_node[sel] - c * PER_CORE] = xf[sel].T

    dcol = np.zeros((NCORES, 128, NBLK), np.float32)
    for c in range(NCORES):
        t0 = c * PER_CORE
        dcol[c] = dinv_tok[t0:t0 + PER_CORE].reshape(NBLK, 128).T.astype(
            np.float32)

    # CA = C*A [64, NPAD]
    cnt = np.bincount(batch, minlength=NGR).astype(np.float64)
    cinv = 1.0 / np.maximum(cnt, 1.0)
    CA = np.zeros((NGR, NPAD))
    np.add.at(CA, (batch[dst], s_tok), dinv_tok[d_tok])
    np.add.at(CA, (batch, tok_of_node), dinv_tok[tok_of_node])
    CA *= dinv_tok[None, :]
    CA *= cinv[:, None]
    ca_t = np.zeros((NCORES, PER_CORE, NGR), np.float32)
    for c in range(NCORES):
        t0 = c * PER_CORE
        ca_t[c] = CA[:, t0:t0 + PER_CORE].T

    # --- gather structure: edges by (dst block, src asc) ---
    eb = d_tok // 128
    eorder = np.lexsort((s_tok, eb))
    s_e, d_e, b_e = s_tok[eorder], d_tok[eorder], eb[eorder]
    bstart = np.searchsorted(b_e, np.arange(nblocks + 1))

    # rounds per (slot, half) unified over cores. The src range
    # [HIBASE, WIN) is reachable from both windows, so the per-block cut
    # can be shifted: align every core's lo-half to a multiple of 128
    # (zero lo padding) at a common per-slot round count.
    cut_minmax = {}
    for gb in range(nblocks):
        c, l = gb // NBLK, gb % NBLK
        lo, hi = bstart[gb], bstart[gb + 1]
        ss = s_e[lo:hi]
        cut_minmax[(c, l)] = (int(np.searchsorted(ss, HIBASE)),
                              int(np.searchsorted(ss, WIN)), lo, hi)
    RLO = np.zeros(NBLK, np.int64)
    RHI = np.zeros(NBLK, np.int64)
    halves = {}   # (core, slot) -> (ss_lo, dd_lo, ss_hi, dd_hi)
    for l in range(NBLK):
        kmax = min(cut_minmax[(c, l)][1] // 128 for c in range(NCORES))
        kmin = max((cut_minmax[(c, l)][0] + 127) // 128
                   for c in range(NCORES))
        for c in range(NCORES):
            cmin, cmax, lo, hi = cut_minmax[(c, l)]
            if kmin <= kmax:
                ncut = min(kmax * 128, cmax)
                if ncut < cmin:      # cannot happen when kmin<=kmax
                    ncut = cmin
            else:
                ncut = int(np.clip((cmin + cmax) // 2, cmin, cmax))
            ss = s_e[lo:hi]
            dd = d_e[lo:hi] % 128
            halves[(c, l)] = (ss[:ncut], dd[:ncut], ss[ncut:], dd[ncut:])
            RLO[l] = max(RLO[l], (ncut + 127) // 128)
            RHI[l] = max(RHI[l], (ss.size - ncut + 127) // 128)

    tot_rounds = int((RLO + RHI).sum())
    lo_off = np.concatenate([[0], np.cumsum(RLO)])            # [NBLK+1]
    LO_TOT = int(lo_off[-1])
    hi_off = LO_TOT + np.concatenate([[0], np.cumsum(RHI)])   # [NBLK+1]

    # build padded idx + one-hot tables per core (lo stream then hi stream)
    idx_cores = np.zeros((NCORES, 16, tot_rounds * 8), np.int16)
    oh_cores = np.zeros((NCORES, tot_rounds, 128, 128), np.float32)
    for c in range(NCORES):
        for l in range(NBLK):
            ss_lo, dd_lo, ss_hi, dd_hi = halves[(c, l)]
            for base, ss_h, dd_h, R, ro in (
                    (0, ss_lo, dd_lo, int(RLO[l]), int(lo_off[l])),
                    (HIBASE, ss_hi, dd_hi, int(RHI[l]), int(hi_off[l]))):
                n = ss_h.size
                idx = np.zeros(R * 128, np.int16)
                idx[:n] = (ss_h - base).astype(np.int16)
                idx_cores[c, :, ro * 8:(ro + R) * 8] = idx.reshape(-1, 16).T
                er = np.arange(n)
                oh_cores[c, ro + er // 128, er % 128, dd_h] = 1.0

    return dict(xt=xt, dcol=dcol, ca=ca_t, dinv_tok=dinv_tok,
                tok_of_node=tok_of_node, idx=idx_cores, oh=oh_cores,
                RLO=RLO, RHI=RHI, lo_off=lo_off, hi_off=hi_off,
                tot_rounds=tot_rounds)


# ---------------------------------------------------------------------------
# device program
# ---------------------------------------------------------------------------

def _build_program(RLO, RHI, lo_off, hi_off, tot_rounds, nonzero_bias):
    import concourse.bacc as bacc
    import concourse.tile as tile
    import concourse.mybir as mybir

    f32 = mybir.dt.float32
    bf16 = mybir.dt.bfloat16
    i16 = mybir.dt.int16
    AF = mybir.ActivationFunctionType
    OP = mybir.AluOpType

    nc = bacc.Bacc("TRN2", target_bir_lowering=False, debug=False,
                   num_devices=NCORES, num_swdge_queues=4)

    xt = nc.dram_tensor("xt", [D, PER_CORE], bf16, kind="ExternalInput")
    idx = nc.dram_tensor("idx", [128, tot_rounds * 8], i16,
                         kind="ExternalInput")
    fp8 = mybir.dt.float8e4
    oh = nc.dram_tensor("oh", [128, tot_rounds * 128], fp8,
                        kind="ExternalInput")
    dcol = nc.dram_tensor("dcol", [128, NBLK], f32, kind="ExternalInput")
    d2col = nc.dram_tensor("d2col", [128, NBLK], f32, kind="ExternalInput")
    ca = nc.dram_tensor("ca", [128, NBLK * NGR], bf16, kind="ExternalInput")
    w1 = nc.dram_tensor("w1", [D, D], bf16, kind="ExternalInput")
    w2 = nc.dram_tensor("w2", [D, D], bf16, kind="ExternalInput")
    w3t = nc.dram_tensor("w3t", [D, D], bf16, kind="ExternalInput")
    linw = nc.dram_tensor("linw", [D, NCLS], bf16, kind="ExternalInput")
    bvec = nc.dram_tensor("bvec", [NCLS, 1], f32, kind="ExternalInput")
    ident = nc.dram_tensor("ident", [128, 128], bf16, kind="ExternalInput")
    b1bc = nc.dram_tensor("b1bc", [128, D], f32, kind="ExternalInput")
    b2bc = nc.dram_tensor("b2bc", [128, D], f32, kind="ExternalInput")

    out_t = nc.dram_tensor("out_t", [NCLS, NGR], f32, kind="ExternalOutput")

    t_loc = [nc.dram_tensor(f"t{i}loc", [PER_CORE, D], bf16, kind="Internal")
             for i in range(2)]
    t_ful = [nc.dram_tensor(f"t{i}ful", [NPAD, D], bf16, kind="Internal",
                            addr_space="Shared") for i in range(2)]
    ar_in = nc.dram_tensor("ar_in", [128, NGR], f32, kind="Internal")
    ar_out = nc.dram_tensor("ar_out", [128, NGR], f32, kind="Internal",
                            addr_space="Shared")

    groups = [list(range(NCORES))]
    qn = [0]

    def next_q():
        q = qn[0] % 4
        qn[0] += 1
        return q

    with tile.TileContext(nc) as tc:
        with tc.tile_pool(name="const", bufs=1) as cp, \
             tc.tile_pool(name="gat", bufs=9) as gp, \
             tc.tile_pool(name="wk", bufs=4) as wp, \
             tc.tile_pool(name="psA", bufs=2, space="PSUM") as pa, \
             tc.tile_pool(name="psT", bufs=2, space="PSUM") as pt, \
             tc.tile_pool(name="psZ", bufs=2, space="PSUM") as pz, \
             tc.tile_pool(name="psP", bufs=1, space="PSUM") as ppool:
            # PSUM budget: psA 2 + psT 2 + psZ 2 + psP 1 = 7 banks

            def load_const(t, shape, dtype):
                tl = cp.tile(list(shape), dtype, tag=t.name)
                nc.sync.dma_start(out=tl[:], in_=t.ap())
                return tl

            xt_t = load_const(xt, [D, PER_CORE], bf16)
            t1all = cp.tile([128, NBLK, 128], bf16, tag="t1all")
            t2all = cp.tile([128, NBLK, 128], bf16, tag="t2all")
            dcol_t = load_const(dcol, [128, NBLK], f32)
            d2col_t = load_const(d2col, [128, NBLK], f32)
            ca_t = load_const(ca, [128, NBLK * NGR], bf16)
            w1_t = load_const(w1, [D, D], bf16)
            w2_t = load_const(w2, [D, D], bf16)
            w3t_t = load_const(w3t, [D, D], bf16)
            linw_t = load_const(linw, [D, NCLS], bf16)
            bvec_t = load_const(bvec, [NCLS, 1], f32)
            id_t = load_const(ident, [128, 128], bf16)
            if nonzero_bias:
                b1bc_t = load_const(b1bc, [128, D], f32)
                b2bc_t = load_const(b2bc, [128, D], f32)

            # Wf = W3 @ lin_w  (lhsT = W3^T)
            wf_ps = pz.tile([128, D], f32, tag="z")
            nc.tensor.matmul(wf_ps[:, 0:NCLS], lhsT=w3t_t[:], rhs=linw_t[:],
                             start=True, stop=True)
            wf_t = cp.tile([D, NCLS], bf16, tag="wf")
            nc.scalar.activation(wf_t[:], wf_ps[:, 0:NCLS], AF.Copy)

            # ---- phase 0: T1' slice ----
            for l in range(NBLK):
                ps0 = pz.tile([128, D], f32, tag="z")
                nc.tensor.matmul(ps0[:], lhsT=xt_t[:, l * 128:(l + 1) * 128],
                                 rhs=w1_t[:], start=True, stop=True)
                nc.scalar.activation(t1all[:, l, :], ps0[:], AF.Copy,
                                     scale=dcol_t[:, l:l + 1])
            nc.sync.dma_start(
                out=t_loc[0].ap().rearrange("(l p) f -> p l f", p=128),
                in_=t1all[:])
            nc.gpsimd.collective_compute(
                "AllGather", mybir.AluOpType.bypass, replica_groups=groups,
                ins=[t_loc[0].ap()], outs=[t_ful[0].ap()])

            # big constant streams: emitted after AG1 so their DMA overlaps
            # the collective and early layer-1 instead of delaying phase 0
            idx_t = load_const(idx, [128, tot_rounds * 8], i16)
            oh_t = load_const(oh, [128, tot_rounds * 128],
                              mybir.dt.float8e4)

            pool_ps = ppool.tile([128, NGR], f32, tag="poolps")

            LO_TOT = int(lo_off[-1])
            TOT = tot_rounds

            for lay in range(2):
                tful = t_ful[lay]
                tloc = t_loc[lay]
                lo_view = tful.ap()[0:WIN, :]
                hi_view = tful.ap()[HIBASE:NPAD, :]
                # fixed-size calls over the two global round streams
                calls = {}     # call key -> (gather tile, oh tile, r0, rt)

                def ensure_call(stream0, stream1, view, ro):
                    k = (ro - stream0) // RCALL
                    key = (stream0, k)
                    if key in calls:
                        return calls[key]
                    r0 = stream0 + k * RCALL
                    rt = min(RCALL, stream1 - r0)
                    gt = gp.tile([128, rt, 128], bf16, tag="gt")
                    nc.gpsimd.dma_gather(
                        gt[:], view, idx_t[:, r0 * 8:(r0 + rt) * 8],
                        num_idxs=rt * 128, num_idxs_reg=rt * 128,
                        elem_size=128, single_packet=False,
                        queue_num=next_q())
                    calls[key] = (gt, r0, rt)
                    return calls[key]

                tall = t1all if lay == 0 else t2all
                for l in range(NBLK):
                    sf = tall[:, l, :]
                    # accumulate
                    ps = pa.tile([128, D], f32, tag="agg")
                    first = True
                    for s0, s1, view, ro0, nr in (
                            (0, LO_TOT, lo_view, int(lo_off[l]), int(RLO[l])),
                            (LO_TOT, TOT, hi_view, int(hi_off[l]),
                             int(RHI[l]))):
                        for r in range(ro0, ro0 + nr):
                            gt, cr0, crt = ensure_call(s0, s1, view, r)
                            j = r - cr0
                            nc.tensor.matmul(
                                ps[:], lhsT=oh_t[:, r * 128:(r + 1) * 128],
                                rhs=gt[:, j, :], start=first, stop=False)
                            first = False
                    nc.tensor.matmul(ps[:], lhsT=id_t[:], rhs=sf,
                                     start=first, stop=True)
                    if lay == 0:
                        # H' = relu(dinv^2 * PSUM)  [dst, hid] bf16
                        h1s = wp.tile([128, D], bf16, tag="h1s")
                        if nonzero_bias:
                            tmp = wp.tile([128, D], f32, tag="tmpb")
                            nc.scalar.activation(tmp[:], ps[:], AF.Copy,
                                                 scale=dcol_t[:, l:l + 1])
                            nc.vector.tensor_tensor(tmp[:], tmp[:], b1bc_t[:],
                                                    OP.add)
                            nc.vector.tensor_relu(tmp[:], tmp[:])
                            nc.scalar.activation(h1s[:], tmp[:], AF.Copy,
                                                 scale=dcol_t[:, l:l + 1])
                        else:
                            nc.scalar.activation(h1s[:], ps[:], AF.Relu,
                                                 scale=d2col_t[:, l:l + 1])
                        # transpose -> [hid, dst]
                        tr = pt.tile([128, D], bf16, tag="tr")
                        nc.tensor.transpose(tr[:], h1s[:], id_t[:])
                        hts = wp.tile([128, D], bf16, tag="hts")
                        nc.scalar.activation(hts[:], tr[:], AF.Copy)
                        # z2T = W2^T @ H'^T  [hid2, dst]
                        z = pz.tile([128, D], f32, tag="z")
                        nc.tensor.matmul(z[:], lhsT=w2_t[:], rhs=hts[:],
                                         start=True, stop=True)
                        zs = wp.tile([128, D], bf16, tag="zs")
                        nc.scalar.activation(zs[:], z[:], AF.Copy)
                        tr2 = pt.tile([128, D], bf16, tag="tr")
                        nc.tensor.transpose(tr2[:], zs[:], id_t[:])
                        nc.scalar.activation(t2all[:, l, :], tr2[:],
                                             AF.Copy)
                        nc.sync.dma_start(
                            out=t_loc[1].ap()[l * 128:(l + 1) * 128, :],
                            in_=t2all[:, l, :])
                    else:
                        # r2 = relu(dinv * PSUM) [dst, hid] bf16
                        r2s = wp.tile([128, D], bf16, tag="r2s")
                        if nonzero_bias:
                            tmp = wp.tile([128, D], f32, tag="tmpb")
                            nc.scalar.activation(tmp[:], ps[:], AF.Copy,
                                                 scale=dcol_t[:, l:l + 1])
                            nc.vector.tensor_tensor(tmp[:], tmp[:], b2bc_t[:],
                                                    OP.add)
                            nc.vector.tensor_relu(tmp[:], tmp[:])
                            nc.scalar.activation(r2s[:], tmp[:], AF.Copy)
                        else:
                            nc.scalar.activation(r2s[:], ps[:], AF.Relu,
                                                 scale=dcol_t[:, l:l + 1])
                        # pooledT += r2^T @ CA_b   [hid, 64]
                        nc.tensor.matmul(
                            pool_ps[:, 0:NGR], lhsT=r2s[:],
                            rhs=ca_t[:, l * NGR:(l + 1) * NGR],
                            start=(l == 0), stop=(l == NBLK - 1))
                if lay == 0:
                    nc.gpsimd.collective_compute(
                        "AllGather", mybir.AluOpType.bypass,
                        replica_groups=groups,
                        ins=[t_loc[1].ap()], outs=[t_ful[1].ap()])

            # ---- epilogue ----
            posb = wp.tile([128, NGR], f32, tag="posb")
            nc.scalar.activation(posb[:], pool_ps[:], AF.Copy)
            nc.sync.dma_start(out=ar_in.ap(), in_=posb[:])
            nc.gpsimd.collective_compute(
                "AllReduce", mybir.AluOpType.add, replica_groups=groups,
                ins=[ar_in.ap()], outs=[ar_out.ap()])
            par32 = wp.tile([128, NGR], f32, tag="par32")
            nc.sync.dma_start(out=par32[:], in_=ar_out.ap())
            par = wp.tile([128, NGR], bf16, tag="par")
            nc.scalar.activation(par[:], par32[:], AF.Copy)
            outp = pz.tile([128, D], f32, tag="z")
            nc.tensor.matmul(outp[0:NCLS, 0:NGR], lhsT=wf_t[:], rhs=par[:],
                             start=True, stop=True)
            outs = wp.tile([NCLS, NGR], f32, tag="outs")
            nc.vector.tensor_scalar(outs[:], outp[0:NCLS, 0:NGR],
                                    bvec_t[:, 0:1], None, OP.add)
            nc.sync.dma_start(out=out_t.ap(), in_=outs[:])

    nc.compile()
    return nc


# ---------------------------------------------------------------------------
# entry point
# ---------------------------------------------------------------------------

def _run(x, edge_index, batch, W1, b1, W2, b2, W3, b3, lin_w, lin_b,
         trace=False):
    from concourse import bass_utils
    import jax.numpy as jnp

    def bf(a):
        return np.asarray(jnp.asarray(np.asarray(a, np.float32),
                                      jnp.bfloat16))

    P = _preprocess(x, edge_index, batch)
    nonzero_bias = bool(np.any(np.asarray(b1)) or np.any(np.asarray(b2)))
    nc = _build_program(P["RLO"], P["RHI"], P["lo_off"], P["hi_off"],
                        P["tot_rounds"], nonzero_bias)

    W1f = np.asarray(W1, np.float32)
    W2f = np.asarray(W2, np.float32)
    W3f = np.asarray(W3, np.float32)
    lwf = np.asarray(lin_w, np.float32)
    bv = (np.asarray(b3, np.float32) @ lwf
          + np.asarray(lin_b, np.float32)).reshape(NCLS, 1)

    shared = {
        "w1": bf(W1f), "w2": bf(W2f), "w3t": bf(W3f.T), "linw": bf(lwf),
        "bvec": bv.astype(np.float32),
        "ident": bf(np.eye(128, dtype=np.float32)),
        "b1bc": np.tile(np.asarray(b1, np.float32), (128, 1)),
        "b2bc": np.tile(np.asarray(b2, np.float32), (128, 1)),
    }
    in_maps = []
    for c in range(NCORES):
        m = dict(shared)
        m["xt"] = bf(P["xt"][c])
        m["idx"] = np.tile(P["idx"][c], (8, 1))
        import ml_dtypes
        m["oh"] = P["oh"][c].transpose(1, 0, 2).reshape(128, -1).astype(
            ml_dtypes.float8_e4m3)
        m["dcol"] = P["dcol"][c]
        m["d2col"] = P["dcol"][c] ** 2
        m["ca"] = bf(P["ca"][c].reshape(NBLK, 128, NGR).transpose(
            1, 0, 2).reshape(128, NBLK * NGR))
        in_maps.append(m)

    res = bass_utils.run_bass_kernel_spmd(
        nc, in_maps, core_ids=list(range(NCORES)), trace=trace)
    out = np.asarray(res.results[0]["out_t"]).T.copy()
    return out, res


def kernel(x, edge_index, batch, W1, b1, W2, b2, W3, b3, lin_w, lin_b):
    out, _ = _run(x, edge_index, batch, W1, b1, W2, b2, W3, b3,
                  lin_w, lin_b, trace=False)
    return out
